# revision 29
# baseline (speedup 1.0000x reference)
"""Trainium2 Bass kernel for nn_NodeAttention (hypergraph message passing).

Math (reference):
    w      = sigmoid(x @ attn_w.T + attn_b)[:, 0]          # per-edge weight (M == N)
    e_feat = Binv * segsum_by_edge(x[node_idx]) @ lin_w.T  # node -> hyperedge
    D      = segsum_by_node(w[edge_idx])
    out    = Dinv * segsum_by_node(e_feat[edge_idx]) + bias

Distribution: 8 cores; core c owns edge rows [c*6250, (c+1)*6250) for the
node->edge phase and node rows of the same range for the edge->node phase.
Each phase is a row gather (SWDGE dma_gather from a replicated DRAM table)
followed by a one-hot-matmul segment sum over windows of 128 destination
segments. lin_w is applied once per 128-row window after aggregation (matmul
commutes with the segment sum); w is carried as column 128 of the intermediate
table so D falls out of the phase-B segment sum for free.

Key performance structure (all verified against perfetto traces):
  - dma_gather descriptor generation runs on one Q7 core pair selected by
    queue_num; rotating calls over 4 SWDGE queues (each with its own
    1024-descriptor ring) removes the ring-full stalls that dominated the
    single-queue version. A call is capped at 1024 descriptors and 4096
    bytes per destination partition (HW-probed).
  - Gather tables are bf16 (x pre-cast on host; the intermediate table is
    written bf16 by phase A), halving HBM gather traffic.
  - Per-(window, half) tile counts are exact per-run compile-time constants
    (max over the 8 cores); index images are padded with index 0 and the
    pad lanes are discarded by their -1 one-hot columns.

dma_gather uses int16 indices, so each table is split at row 32768 into lo/hi
halves. Host-side work is limited to index preprocessing (partition by
destination, sort, pad), hyperedge degree counts, and a dtype cast of x; all
x-dependent math runs on device.
"""

import os
import sys
from contextlib import ExitStack

import numpy as np

for _p in (
    "/root/.axon_site",
    "/root/.axon_site/_ro/trn_rl_repo",
    "/root/.axon_site/_ro/pypackages",
):
    if os.path.isdir(_p) and _p not in sys.path:
        sys.path.append(_p)

import concourse.bass as bass
import concourse.mybir as mybir
import concourse.tile as tile
from concourse import bacc
from concourse.bass_utils import run_bass_kernel_spmd
from concourse.masks import make_identity

P = 128
N_NODES = 50000
N_EDGES = 50000
C = 128          # feature channels
CT = 256         # intermediate bf16 row: [e_feat(128) | w(1) | pad], 512 bytes
HALF = 32768     # int16 index split point
NCORES = 8
SLAB = N_NODES // NCORES          # 6250 rows owned per core
WPC = (SLAB + P - 1) // P         # 49 windows of 128 destinations per core
NQ = 4                            # SWDGE queues, round-robin per gather call

F32 = mybir.dt.float32
BF16 = mybir.dt.bfloat16
I16 = mybir.dt.int16

# Set by test harness to capture NTFF profiles / exec times.
TRACE = False
LAST_EXEC_NS = {}

_PROGRAMS = {}


# ----------------------------------------------------------------------------
# Host-side index preprocessing
# ----------------------------------------------------------------------------

def _plan_phase(dst_ids, src_ids):
    """Group entries by (destination core, 128-dest window, src half); per
    (window, half) the tile count is the max over cores of ceil(count/128)
    and each core's index image is 0-padded past its own count.

    Returns (t_lo, t_hi, img_lo, img_hi, dst):
      t_lo/t_hi: [WPC] int tile counts per window for the lo/hi halves
      img_lo:    [NCORES, P, sum(t_lo) * 8] int16 dma_gather index image
      img_hi:    [NCORES, P, sum(t_hi) * 8] int16 (indices rebased by -HALF)
      dst:       [NCORES, P, sum(t_lo) + sum(t_hi)] bf16 dest-rel, pad -1
    Within window w the gather sequence is lo entries then hi entries; entry
    i lands at SBUF partition i%128 of tile i//128 of its half, and its
    one-hot column lives at dst[:, tile_col].
    """
    dst_ids = np.asarray(dst_ids, np.int64)
    src_ids = np.asarray(src_ids, np.int64)
    core = dst_ids // SLAB
    local = dst_ids - core * SLAB
    w = local // P
    rel = (local - w * P).astype(np.float32)
    hi = (src_ids >= HALF).astype(np.int64)
    key = (core * WPC + w) * 2 + hi
    order = np.argsort(key, kind="stable")
    k = key[order]
    s = src_ids[order]
    r = rel[order]
    n_grp = NCORES * WPC * 2
    counts = np.bincount(k, minlength=n_grp).reshape(NCORES, WPC, 2)
    t_lo = np.ceil(counts[:, :, 0].max(axis=0) / P).astype(np.int64)  # [WPC]
    t_hi = np.ceil(counts[:, :, 1].max(axis=0) / P).astype(np.int64)
    starts = np.cumsum(counts.reshape(-1)) - counts.reshape(-1)
    rank = np.arange(k.shape[0], dtype=np.int64) - starts[k]
    half_flag = k % 2
    gw = k // 2
    cc = gw // WPC
    ww = gw - cc * WPC

    # capacity layout: per half, window w owns tiles [tbase[w], tbase[w]+t[w]).
    # Bases are rounded to even so every dma_gather index-image slice starts
    # 32-byte aligned; the pad tiles between windows are never referenced.
    cap_lo = ((t_lo + 1) // 2) * 2
    cap_hi = ((t_hi + 1) // 2) * 2
    tbase_lo = np.concatenate([[0], np.cumsum(cap_lo)[:-1]])
    tbase_hi = np.concatenate([[0], np.cumsum(cap_hi)[:-1]])
    ntile_lo = int(cap_lo.sum())
    ntile_hi = int(cap_hi.sum())

    def build_img(sel, tb, ntiles, base):
        # Pad with index 0: trailing -1 "trim" entries abort on HW at scale
        # (probed); row-0 pad gathers are discarded by the -1 one-hot columns.
        cap = ntiles * P
        img_seq = np.zeros((NCORES, cap), np.int16)
        pos = (tb[ww[sel]] * P + rank[sel]).astype(np.int64)
        img_seq[cc[sel], pos] = (s[sel] - base).astype(np.int16)
        # index i -> partition i % 16, column i // 16; replicate x8 partitions
        img = img_seq.reshape(NCORES, cap // 16, 16).transpose(0, 2, 1)
        return np.ascontiguousarray(np.tile(img, (1, 8, 1)))

    img_lo = build_img(half_flag == 0, tbase_lo, ntile_lo, 0)
    img_hi = build_img(half_flag == 1, tbase_hi, ntile_hi, HALF)

    # dst one-hot image: one column per tile (lo tiles then hi tiles, in
    # window-major concatenated layout to match the kernel's column indexing)
    t_tot = t_lo + t_hi
    tbase_tot = np.concatenate([[0], np.cumsum(t_tot)[:-1]])
    dst_img = np.full((NCORES, P, int(t_tot.sum())), -1.0, np.float32)
    lane = rank % P
    tloc = rank // P
    col = np.where(
        half_flag == 0,
        tbase_tot[ww] + tloc,
        tbase_tot[ww] + t_lo[ww] + tloc,
    )
    dst_img[cc, lane, col] = r
    return t_lo, t_hi, img_lo, img_hi, dst_img


# ----------------------------------------------------------------------------
# Bass programs
# ----------------------------------------------------------------------------

def _new_nc():
    return bacc.Bacc(
        "TRN2",
        target_bir_lowering=False,
        debug=False,
        enable_asserts=False,
        num_devices=NCORES,
        num_swdge_queues=NQ,
    )


class _QueueRR:
    def __init__(self):
        self.i = 0

    def next(self):
        q = self.i % NQ
        self.i += 1
        return q


# Per dma_gather call (HW-probed): <= 1024 descriptors AND <= 4096 bytes
# written per destination partition.
MAX_CALL_TILES = 8


def _phase_a_program(t_lo, t_hi):
    """Node->edge aggregation, producing the per-core slab of the
    intermediate table ea[slab, CT] = [Binv * segsum(x rows) @ lin_w.T | w]."""
    t_lo = list(map(int, t_lo))
    t_hi = list(map(int, t_hi))
    cap_lo = [(t + 1) // 2 * 2 for t in t_lo]
    cap_hi = [(t + 1) // 2 * 2 for t in t_hi]
    ntl, nth = sum(cap_lo), sum(cap_hi)
    ntt = sum(t_lo) + sum(t_hi)
    tb_lo = np.concatenate([[0], np.cumsum(cap_lo)[:-1]]).astype(int)
    tb_hi = np.concatenate([[0], np.cumsum(cap_hi)[:-1]]).astype(int)
    tb_tot = np.concatenate([[0], np.cumsum(np.add(t_lo, t_hi))[:-1]]).astype(int)
    tmax = max(tl + th for tl, th in zip(t_lo, t_hi))

    nc = _new_nc()
    xbf = nc.dram_tensor("xbf", [N_NODES, C], BF16, kind="ExternalInput").ap()
    xslab = nc.dram_tensor("xslab", [WPC * P, C], F32, kind="ExternalInput").ap()
    ilo = nc.dram_tensor("ilo", [P, ntl * 8], I16, kind="ExternalInput").ap()
    ihi = nc.dram_tensor("ihi", [P, nth * 8], I16, kind="ExternalInput").ap()
    dst = nc.dram_tensor("dst", [P, ntt], F32, kind="ExternalInput").ap()
    binv = nc.dram_tensor("binv", [P, WPC], F32, kind="ExternalInput").ap()
    wt = nc.dram_tensor("wt", [C, C], F32, kind="ExternalInput").ap()
    arep = nc.dram_tensor("arep", [P, C], F32, kind="ExternalInput").ap()
    bcol = nc.dram_tensor("bcol", [P, 1], F32, kind="ExternalInput").ap()
    eslab = nc.dram_tensor("eslab", [SLAB, CT], BF16, kind="ExternalOutput").ap()
    warm = nc.dram_tensor("warm", [P, C], BF16, kind="ExternalOutput").ap()

    qrr = _QueueRR()
    with tile.TileContext(nc) as tc:
        with ExitStack() as ctx:
            const = ctx.enter_context(tc.tile_pool(name="const", bufs=1))
            gpool = ctx.enter_context(tc.tile_pool(name="gather", bufs=4))
            spool = ctx.enter_context(tc.tile_pool(name="onehot", bufs=6))
            wpool = ctx.enter_context(tc.tile_pool(name="work", bufs=3))
            opool = ctx.enter_context(tc.tile_pool(name="out", bufs=3))
            pseg = ctx.enter_context(tc.tile_pool(name="pseg", bufs=2, space="PSUM"))
            ptr = ctx.enter_context(tc.tile_pool(name="ptr", bufs=2, space="PSUM"))
            pout = ctx.enter_context(tc.tile_pool(name="pout", bufs=2, space="PSUM"))

            # index images first: the gathers depend only on these
            ilo_sb = const.tile([P, ntl * 8], I16)
            nc.sync.dma_start(out=ilo_sb[:], in_=ilo[:])
            ihi_sb = const.tile([P, nth * 8], I16)
            nc.sync.dma_start(out=ihi_sb[:], in_=ihi[:])

            # warmup gathers: absorb the dma_gather ucode IRAM load and ring
            # init on every queue while the input DMAs stream in.
            wimg = const.tile([P, 8], I16)
            nc.vector.memset(wimg[:], 0)
            wg = const.tile([P, C], BF16)
            for q in range(NQ):
                nc.gpsimd.dma_gather(
                    wg[:].rearrange("p (t c) -> p t c", c=C),
                    xbf[:HALF, :], wimg[:], P, P, C, queue_num=q,
                )
            nc.sync.dma_start(out=warm[:], in_=wg[:])

            ident = const.tile([P, P], F32)
            make_identity(nc, ident[:])
            iota_i = const.tile([P, P], mybir.dt.int32)
            nc.gpsimd.iota(iota_i[:], pattern=[[1, P]], base=0, channel_multiplier=0)
            iota_f = const.tile([P, P], F32)
            nc.vector.tensor_copy(iota_f[:], iota_i[:])

            wt_sb = const.tile([C, C], F32)
            nc.sync.dma_start(out=wt_sb[:], in_=wt[:])
            a_sb = const.tile([P, C], F32)
            nc.sync.dma_start(out=a_sb[:], in_=arep[:])
            b_sb = const.tile([P, 1], F32)
            nc.sync.dma_start(out=b_sb[:], in_=bcol[:])
            dst_sb = const.tile([P, ntt], F32)
            nc.sync.dma_start(out=dst_sb[:], in_=dst[:])
            binv_sb = const.tile([P, WPC], F32)
            nc.sync.dma_start(out=binv_sb[:], in_=binv[:])

            # slab rows of x, window-major: xsl[p, w*C + c] = xslab[w*128 + p, c]
            xsl = const.tile([P, WPC * C], F32)
            nc.sync.dma_start(
                out=xsl[:].rearrange("p (w c) -> p w c", c=C),
                in_=xslab.rearrange("(w p) c -> p w c", p=P),
            )

            # attention scores for the slab: w = sigmoid(x . a + b), one col/window
            wraw = const.tile([P, WPC], F32)
            for w in range(WPC):
                prod = wpool.tile([P, C], F32, tag="prod")
                nc.vector.tensor_tensor(
                    prod[:], xsl[:, w * C : (w + 1) * C], a_sb[:],
                    op=mybir.AluOpType.mult,
                )
                nc.vector.tensor_reduce(
                    wraw[:, w : w + 1], prod[:],
                    axis=mybir.AxisListType.X, op=mybir.AluOpType.add,
                )
            wall = const.tile([P, WPC], F32)
            nc.scalar.activation(
                wall[:], wraw[:], mybir.ActivationFunctionType.Sigmoid,
                bias=b_sb[:, 0:1], scale=1.0,
            )

            # zero-fill gather buffers once: trimmed lanes are never written
            # by the DMA, and one-hot zero columns must multiply finite data.
            for _i in range(4):
                gz = gpool.tile([P, tmax * C], BF16, tag="g", name=f"gz{_i}")
                nc.vector.memset(gz[:], 0.0)

            for w in range(WPC):
                rows = min(P, SLAB - w * P)
                tl, th = t_lo[w], t_hi[w]
                tt = tl + th
                g = gpool.tile([P, tmax * C], BF16, tag="g")
                off = 0
                for img, tb, t_half in ((ilo_sb, tb_lo[w], tl), (ihi_sb, tb_hi[w], th)):
                    tab = xbf[:HALF, :] if img is ilo_sb else xbf[HALF:, :]
                    t0 = 0
                    while t0 < t_half:
                        tn = min(MAX_CALL_TILES, t_half - t0)
                        ni = tn * P
                        nc.gpsimd.dma_gather(
                            g[:, (off + t0) * C : (off + t0 + tn) * C].rearrange(
                                "p (t c) -> p t c", c=C
                            ),
                            tab,
                            img[:, (tb + t0) * 8 : (tb + t0 + tn) * 8],
                            ni,
                            ni,
                            C,
                            queue_num=w % NQ,
                        )
                        t0 += tn
                    off += t_half
                ps = pseg.tile([P, C], F32)
                for t in range(tt):
                    col = tb_tot[w] + t
                    s_t = spool.tile([P, P], BF16, tag="S")
                    nc.vector.tensor_tensor(
                        s_t[:],
                        dst_sb[:, col : col + 1].to_broadcast([P, P]),
                        iota_f[:],
                        op=mybir.AluOpType.is_equal,
                    )
                    nc.tensor.matmul(
                        out=ps[:], lhsT=s_t[:], rhs=g[:, t * C : (t + 1) * C],
                        start=(t == 0), stop=(t == tt - 1),
                    )
                # scale rows by Binv while draining PSUM
                epre = wpool.tile([P, C], F32, tag="epre")
                nc.scalar.activation(
                    epre[:], ps[:], mybir.ActivationFunctionType.Copy,
                    scale=binv_sb[:, w : w + 1],
                )
                pst = ptr.tile([P, P], F32)
                nc.tensor.transpose(pst[:], epre[:], ident[:])
                epret = wpool.tile([P, P], F32, tag="epret")
                nc.scalar.copy(epret[:], pst[:])
                pso = pout.tile([P, C], F32)
                nc.tensor.matmul(
                    out=pso[:], lhsT=epret[:], rhs=wt_sb[:], start=True, stop=True
                )
                ot = opool.tile([P, CT], BF16, tag="ot")
                nc.scalar.copy(ot[:, 0:C], pso[:])
                nc.vector.tensor_copy(ot[:, C : C + 1], wall[:, w : w + 1])
                nc.vector.memset(ot[:, C + 1 : CT], 0.0)
                nc.sync.dma_start(
                    out=eslab[w * P : w * P + rows, :], in_=ot[:rows, :]
                )
    nc.compile()
    return nc


def _phase_b_program(t_lo, t_hi):
    """Edge->node aggregation over the full intermediate table, producing the
    per-core output slab out[slab, C] = Dinv * segsum(ea rows)[:, :C] + bias."""
    t_lo = list(map(int, t_lo))
    t_hi = list(map(int, t_hi))
    cap_lo = [(t + 1) // 2 * 2 for t in t_lo]
    cap_hi = [(t + 1) // 2 * 2 for t in t_hi]
    ntl, nth = sum(cap_lo), sum(cap_hi)
    ntt = sum(t_lo) + sum(t_hi)
    tb_lo = np.concatenate([[0], np.cumsum(cap_lo)[:-1]]).astype(int)
    tb_hi = np.concatenate([[0], np.cumsum(cap_hi)[:-1]]).astype(int)
    tb_tot = np.concatenate([[0], np.cumsum(np.add(t_lo, t_hi))[:-1]]).astype(int)
    tmax = max(tl + th for tl, th in zip(t_lo, t_hi))

    nc = _new_nc()
    ea = nc.dram_tensor("ea", [N_EDGES, CT], BF16, kind="ExternalInput").ap()
    ilo = nc.dram_tensor("ilo", [P, ntl * 8], I16, kind="ExternalInput").ap()
    ihi = nc.dram_tensor("ihi", [P, nth * 8], I16, kind="ExternalInput").ap()
    dst = nc.dram_tensor("dst", [P, ntt], F32, kind="ExternalInput").ap()
    biasr = nc.dram_tensor("biasr", [P, C], F32, kind="ExternalInput").ap()
    outslab = nc.dram_tensor("outslab", [SLAB, C], F32, kind="ExternalOutput").ap()
    warm = nc.dram_tensor("warm", [P, C], BF16, kind="ExternalOutput").ap()

    qrr = _QueueRR()
    with tile.TileContext(nc) as tc:
        with ExitStack() as ctx:
            const = ctx.enter_context(tc.tile_pool(name="const", bufs=1))
            gpool = ctx.enter_context(tc.tile_pool(name="gather", bufs=4))
            spool = ctx.enter_context(tc.tile_pool(name="onehot", bufs=6))
            wpool = ctx.enter_context(tc.tile_pool(name="work", bufs=3))
            opool = ctx.enter_context(tc.tile_pool(name="out", bufs=3))
            pseg = ctx.enter_context(tc.tile_pool(name="pseg", bufs=2, space="PSUM"))

            ilo_sb = const.tile([P, ntl * 8], I16)
            nc.sync.dma_start(out=ilo_sb[:], in_=ilo[:])
            ihi_sb = const.tile([P, nth * 8], I16)
            nc.sync.dma_start(out=ihi_sb[:], in_=ihi[:])

            wimg = const.tile([P, 8], I16)
            nc.vector.memset(wimg[:], 0)
            wg = const.tile([P, CT], BF16)
            for q in range(NQ):
                nc.gpsimd.dma_gather(
                    wg[:].rearrange("p (t c) -> p t c", c=CT),
                    ea[:HALF, :], wimg[:], P, P, CT, queue_num=q,
                )
            nc.sync.dma_start(out=warm[:], in_=wg[:, :C])

            iota_i = const.tile([P, P], mybir.dt.int32)
            nc.gpsimd.iota(iota_i[:], pattern=[[1, P]], base=0, channel_multiplier=0)
            iota_f = const.tile([P, P], F32)
            nc.vector.tensor_copy(iota_f[:], iota_i[:])

            bias_sb = const.tile([P, C], F32)
            nc.sync.dma_start(out=bias_sb[:], in_=biasr[:])
            dst_sb = const.tile([P, ntt], F32)
            nc.sync.dma_start(out=dst_sb[:], in_=dst[:])

            for _i in range(4):
                gz = gpool.tile([P, tmax * CT], BF16, tag="g", name=f"gz{_i}")
                nc.vector.memset(gz[:], 0.0)

            for w in range(WPC):
                rows = min(P, SLAB - w * P)
                tl, th = t_lo[w], t_hi[w]
                tt = tl + th
                g = gpool.tile([P, tmax * CT], BF16, tag="g")
                off = 0
                for img, tb, t_half in ((ilo_sb, tb_lo[w], tl), (ihi_sb, tb_hi[w], th)):
                    tab = ea[:HALF, :] if img is ilo_sb else ea[HALF:, :]
                    t0 = 0
                    while t0 < t_half:
                        tn = min(MAX_CALL_TILES, t_half - t0)
                        ni = tn * P
                        nc.gpsimd.dma_gather(
                            g[:, (off + t0) * CT : (off + t0 + tn) * CT].rearrange(
                                "p (t c) -> p t c", c=CT
                            ),
                            tab,
                            img[:, (tb + t0) * 8 : (tb + t0 + tn) * 8],
                            ni,
                            ni,
                            CT,
                            queue_num=w % NQ,
                        )
                        t0 += tn
                    off += t_half
                ps = pseg.tile([P, C + 4], F32)
                for t in range(tt):
                    col = tb_tot[w] + t
                    s_t = spool.tile([P, P], BF16, tag="S")
                    nc.vector.tensor_tensor(
                        s_t[:],
                        dst_sb[:, col : col + 1].to_broadcast([P, P]),
                        iota_f[:],
                        op=mybir.AluOpType.is_equal,
                    )
                    nc.tensor.matmul(
                        out=ps[:], lhsT=s_t[:], rhs=g[:, t * CT : t * CT + C + 4],
                        start=(t == 0), stop=(t == tt - 1),
                    )
                # Dinv = 1 / max(D, tiny); zero-degree rows have zero sums so
                # huge * 0 = 0 matches the reference's where(D > 0, 1/D, 0).
                dmax = wpool.tile([P, 1], F32, tag="dmax")
                nc.vector.tensor_scalar_max(dmax[:], ps[:, C : C + 1], 1e-30)
                dinv = wpool.tile([P, 1], F32, tag="dinv")
                nc.vector.reciprocal(dinv[:], dmax[:])
                ot = opool.tile([P, C], F32, tag="ot")
                nc.scalar.activation(
                    ot[:], ps[:, 0:C], mybir.ActivationFunctionType.Copy,
                    scale=dinv[:, 0:1],
                )
                nc.vector.tensor_tensor(
                    ot[:], ot[:], bias_sb[:], op=mybir.AluOpType.add
                )
                nc.sync.dma_start(
                    out=outslab[w * P : w * P + rows, :], in_=ot[:rows, :]
                )
    nc.compile()
    return nc


def _program(phase, t_lo, t_hi):
    key = (phase, tuple(t_lo), tuple(t_hi))
    if key not in _PROGRAMS:
        _PROGRAMS[key] = (
            _phase_a_program(t_lo, t_hi)
            if phase == "A"
            else _phase_b_program(t_lo, t_hi)
        )
    return _PROGRAMS[key]


# ----------------------------------------------------------------------------
# Entry point
# ----------------------------------------------------------------------------

def _run(nc, in_maps, label):
    kwargs = {}
    if TRACE:
        kwargs = dict(trace=True, trace_cores=[0])
    res = run_bass_kernel_spmd(nc, in_maps, core_ids=list(range(NCORES)), **kwargs)
    if res.exec_time_ns is not None:
        LAST_EXEC_NS[label] = res.exec_time_ns
    return res.results


def kernel(x, hyperedge_index, attn_w, attn_b, lin_w, bias):
    from ml_dtypes import bfloat16

    x = np.ascontiguousarray(np.asarray(x, dtype=np.float32))
    he = np.asarray(hyperedge_index)
    node_idx = he[0].astype(np.int64)
    edge_idx = he[1].astype(np.int64)
    attn_w = np.asarray(attn_w, dtype=np.float32)
    attn_b = np.asarray(attn_b, dtype=np.float32)
    lin_w = np.asarray(lin_w, dtype=np.float32)
    bias = np.asarray(bias, dtype=np.float32)

    # --- host index preprocessing ------------------------------------------
    a_lo, a_hi, a_img_lo, a_img_hi, a_dst = _plan_phase(edge_idx, node_idx)
    b_lo, b_hi, b_img_lo, b_img_hi, b_dst = _plan_phase(node_idx, edge_idx)

    bdeg = np.bincount(edge_idx, minlength=N_EDGES).astype(np.float32)
    binv_full = np.where(bdeg > 0, 1.0 / np.maximum(bdeg, 1.0), 0.0).astype(
        np.float32
    )
    pad = WPC * P - SLAB
    binv_cores = np.pad(
        binv_full.reshape(NCORES, SLAB), ((0, 0), (0, pad))
    ).reshape(NCORES, WPC, P).transpose(0, 2, 1)  # [NCORES, P, WPC]
    binv_cores = np.ascontiguousarray(binv_cores)

    wt_host = np.ascontiguousarray(lin_w.T)  # [in_ch, out_ch]
    a_rep = np.ascontiguousarray(np.broadcast_to(attn_w.reshape(1, C), (P, C)))
    b_col = np.full((P, 1), float(attn_b.reshape(-1)[0]), np.float32)
    bias_rep = np.ascontiguousarray(np.broadcast_to(bias.reshape(1, C), (P, C)))

    x_bf = np.ascontiguousarray(x.astype(bfloat16))
    xslab_pad = np.zeros((NCORES, WPC * P, C), np.float32)
    xslab_pad[:, :SLAB] = x.reshape(NCORES, SLAB, C)

    # --- phase A: node -> edge ---------------------------------------------
    nc_a = _program("A", a_lo, a_hi)
    in_maps_a = [
        {
            "xbf": x_bf,
            "xslab": xslab_pad[c],
            "ilo": a_img_lo[c],
            "ihi": a_img_hi[c],
            "dst": a_dst[c],
            "binv": binv_cores[c],
            "wt": wt_host,
            "arep": a_rep,
            "bcol": b_col,
        }
        for c in range(NCORES)
    ]
    res_a = _run(nc_a, in_maps_a, "A")
    ea = np.ascontiguousarray(
        np.concatenate([r["eslab"] for r in res_a], axis=0)
    )  # [N_EDGES, CT] bf16

    # --- phase B: edge -> node ---------------------------------------------
    nc_b = _program("B", b_lo, b_hi)
    in_maps_b = [
        {
            "ea": ea,
            "ilo": b_img_lo[c],
            "ihi": b_img_hi[c],
            "dst": b_dst[c],
            "biasr": bias_rep,
        }
        for c in range(NCORES)
    ]
    res_b = _run(nc_b, in_maps_b, "B")
    out = np.concatenate([r["outslab"] for r in res_b], axis=0)
    return np.ascontiguousarray(out.astype(np.float32))


# revision 30
# speedup vs baseline: 1.2222x; 1.2222x over previous
"""Trainium2 Bass kernel for nn_NodeAttention (hypergraph message passing).

Math (reference):
    w      = sigmoid(x @ attn_w.T + attn_b)[:, 0]          # per-edge weight (M == N)
    e_feat = Binv * segsum_by_edge(x[node_idx]) @ lin_w.T  # node -> hyperedge
    D      = segsum_by_node(w[edge_idx])
    out    = Dinv * segsum_by_node(e_feat[edge_idx]) + bias

Distribution: 8 cores; core c owns edge rows [c*6250, (c+1)*6250) for the
node->edge phase and node rows of the same range for the edge->node phase.
Each phase is a row gather (SWDGE dma_gather from a replicated DRAM table)
followed by a one-hot-matmul segment sum over windows of 128 destination
segments. lin_w is applied once per 128-row window after aggregation (matmul
commutes with the segment sum); w is carried as column 128 of the intermediate
table so D falls out of the phase-B segment sum for free.

Key performance structure (all verified against perfetto traces):
  - dma_gather descriptor generation runs on one Q7 core pair selected by
    queue_num; rotating calls over 4 SWDGE queues (each with its own
    1024-descriptor ring) removes the ring-full stalls that dominated the
    single-queue version. A call is capped at 1024 descriptors and 4096
    bytes per destination partition (HW-probed).
  - Gather tables are bf16 (x pre-cast on host; the intermediate table is
    written bf16 by phase A), halving HBM gather traffic.
  - Per-(window, half) tile counts are exact per-run compile-time constants
    (max over the 8 cores); index images are padded with index 0 and the
    pad lanes are discarded by their -1 one-hot columns.

dma_gather uses int16 indices, so each table is split at row 32768 into lo/hi
halves. Host-side work is limited to index preprocessing (partition by
destination, sort, pad), hyperedge degree counts, and a dtype cast of x; all
x-dependent math runs on device.
"""

import os
import sys
from contextlib import ExitStack

import numpy as np

for _p in (
    "/root/.axon_site",
    "/root/.axon_site/_ro/trn_rl_repo",
    "/root/.axon_site/_ro/pypackages",
):
    if os.path.isdir(_p) and _p not in sys.path:
        sys.path.append(_p)

import concourse.bass as bass
import concourse.mybir as mybir
import concourse.tile as tile
from concourse import bacc
from concourse.bass_utils import run_bass_kernel_spmd
from concourse.masks import make_identity

P = 128
N_NODES = 50000
N_EDGES = 50000
C = 128          # feature channels
CT = 256         # intermediate bf16 row: [e_feat(128) | w(1) | pad], 512 bytes
HALF = 32768     # int16 index split point
NCORES = 8
SLAB = N_NODES // NCORES          # 6250 rows owned per core
WPC = (SLAB + P - 1) // P         # 49 windows of 128 destinations per core
NQ = 4                            # SWDGE queues, round-robin per gather call

F32 = mybir.dt.float32
BF16 = mybir.dt.bfloat16
I16 = mybir.dt.int16

# Set by test harness to capture NTFF profiles / exec times.
TRACE = False
LAST_EXEC_NS = {}

_PROGRAMS = {}


# ----------------------------------------------------------------------------
# Host-side index preprocessing
# ----------------------------------------------------------------------------

def _plan_phase(dst_ids, src_ids):
    """Group entries by (destination core, 128-dest window, src half); per
    (window, half) the tile count is the max over cores of ceil(count/128)
    and each core's index image is 0-padded past its own count.

    Returns (t_lo, t_hi, img_lo, img_hi, dst):
      t_lo/t_hi: [WPC] int tile counts per window for the lo/hi halves
      img_lo:    [NCORES, P, sum(t_lo) * 8] int16 dma_gather index image
      img_hi:    [NCORES, P, sum(t_hi) * 8] int16 (indices rebased by -HALF)
      dst:       [NCORES, P, sum(t_lo) + sum(t_hi)] bf16 dest-rel, pad -1
    Within window w the gather sequence is lo entries then hi entries; entry
    i lands at SBUF partition i%128 of tile i//128 of its half, and its
    one-hot column lives at dst[:, tile_col].
    """
    dst_ids = np.asarray(dst_ids, np.int64)
    src_ids = np.asarray(src_ids, np.int64)
    core = dst_ids // SLAB
    local = dst_ids - core * SLAB
    w = local // P
    rel = (local - w * P).astype(np.float32)
    hi = (src_ids >= HALF).astype(np.int64)
    key = (core * WPC + w) * 2 + hi
    order = np.argsort(key, kind="stable")
    k = key[order]
    s = src_ids[order]
    r = rel[order]
    n_grp = NCORES * WPC * 2
    counts = np.bincount(k, minlength=n_grp).reshape(NCORES, WPC, 2)
    t_lo = np.ceil(counts[:, :, 0].max(axis=0) / P).astype(np.int64)  # [WPC]
    t_hi = np.ceil(counts[:, :, 1].max(axis=0) / P).astype(np.int64)
    starts = np.cumsum(counts.reshape(-1)) - counts.reshape(-1)
    rank = np.arange(k.shape[0], dtype=np.int64) - starts[k]
    half_flag = k % 2
    gw = k // 2
    cc = gw // WPC
    ww = gw - cc * WPC

    # capacity layout: per half, window w owns tiles [tbase[w], tbase[w]+t[w]).
    # Bases are rounded to even so every dma_gather index-image slice starts
    # 32-byte aligned; the pad tiles between windows are never referenced.
    cap_lo = ((t_lo + 1) // 2) * 2
    cap_hi = ((t_hi + 1) // 2) * 2
    tbase_lo = np.concatenate([[0], np.cumsum(cap_lo)[:-1]])
    tbase_hi = np.concatenate([[0], np.cumsum(cap_hi)[:-1]])
    ntile_lo = int(cap_lo.sum())
    ntile_hi = int(cap_hi.sum())

    def build_img(sel, tb, ntiles, base):
        # Pad with index 0: trailing -1 "trim" entries abort on HW at scale
        # (probed); row-0 pad gathers are discarded by the -1 one-hot columns.
        cap = ntiles * P
        img_seq = np.zeros((NCORES, cap), np.int16)
        pos = (tb[ww[sel]] * P + rank[sel]).astype(np.int64)
        img_seq[cc[sel], pos] = (s[sel] - base).astype(np.int16)
        # index i -> partition i % 16, column i // 16; replicate x8 partitions
        img = img_seq.reshape(NCORES, cap // 16, 16).transpose(0, 2, 1)
        return np.ascontiguousarray(np.tile(img, (1, 8, 1)))

    img_lo = build_img(half_flag == 0, tbase_lo, ntile_lo, 0)
    img_hi = build_img(half_flag == 1, tbase_hi, ntile_hi, HALF)

    # dst one-hot image: one column per tile (lo tiles then hi tiles, in
    # window-major concatenated layout to match the kernel's column indexing)
    t_tot = t_lo + t_hi
    tbase_tot = np.concatenate([[0], np.cumsum(t_tot)[:-1]])
    dst_img = np.full((NCORES, P, int(t_tot.sum())), -1.0, np.float32)
    lane = rank % P
    tloc = rank // P
    col = np.where(
        half_flag == 0,
        tbase_tot[ww] + tloc,
        tbase_tot[ww] + t_lo[ww] + tloc,
    )
    dst_img[cc, lane, col] = r
    return t_lo, t_hi, img_lo, img_hi, dst_img


# ----------------------------------------------------------------------------
# Bass programs
# ----------------------------------------------------------------------------

def _new_nc():
    return bacc.Bacc(
        "TRN2",
        target_bir_lowering=False,
        debug=False,
        enable_asserts=False,
        num_devices=NCORES,
        num_swdge_queues=NQ,
    )


class _QueueRR:
    def __init__(self):
        self.i = 0

    def next(self):
        q = self.i % NQ
        self.i += 1
        return q


# Per dma_gather call (HW-probed): <= 1024 descriptors AND <= 4096 bytes
# written per destination partition.
MAX_CALL_TILES = 8


def _phase_a_program(t_lo, t_hi):
    """Node->edge aggregation, producing the per-core slab of the
    intermediate table ea[slab, CT] = [Binv * segsum(x rows) @ lin_w.T | w]."""
    t_lo = list(map(int, t_lo))
    t_hi = list(map(int, t_hi))
    cap_lo = [(t + 1) // 2 * 2 for t in t_lo]
    cap_hi = [(t + 1) // 2 * 2 for t in t_hi]
    ntl, nth = sum(cap_lo), sum(cap_hi)
    ntt = sum(t_lo) + sum(t_hi)
    tb_lo = np.concatenate([[0], np.cumsum(cap_lo)[:-1]]).astype(int)
    tb_hi = np.concatenate([[0], np.cumsum(cap_hi)[:-1]]).astype(int)
    tb_tot = np.concatenate([[0], np.cumsum(np.add(t_lo, t_hi))[:-1]]).astype(int)
    tmax = max(tl + th for tl, th in zip(t_lo, t_hi))

    nc = _new_nc()
    xbf = nc.dram_tensor("xbf", [N_NODES, C], BF16, kind="ExternalInput").ap()
    xslab = nc.dram_tensor("xslab", [WPC * P, C], F32, kind="ExternalInput").ap()
    ilo = nc.dram_tensor("ilo", [P, ntl * 8], I16, kind="ExternalInput").ap()
    ihi = nc.dram_tensor("ihi", [P, nth * 8], I16, kind="ExternalInput").ap()
    dst = nc.dram_tensor("dst", [P, ntt], F32, kind="ExternalInput").ap()
    binv = nc.dram_tensor("binv", [P, WPC], F32, kind="ExternalInput").ap()
    wt = nc.dram_tensor("wt", [C, C], F32, kind="ExternalInput").ap()
    arep = nc.dram_tensor("arep", [P, C], F32, kind="ExternalInput").ap()
    bcol = nc.dram_tensor("bcol", [P, 1], F32, kind="ExternalInput").ap()
    eslab = nc.dram_tensor("eslab", [SLAB, CT], BF16, kind="ExternalOutput").ap()

    qrr = _QueueRR()
    with tile.TileContext(nc) as tc:
        with ExitStack() as ctx:
            const = ctx.enter_context(tc.tile_pool(name="const", bufs=1))
            gpool = ctx.enter_context(tc.tile_pool(name="gather", bufs=4))
            spool = ctx.enter_context(tc.tile_pool(name="onehot", bufs=6))
            wpool = ctx.enter_context(tc.tile_pool(name="work", bufs=3))
            opool = ctx.enter_context(tc.tile_pool(name="out", bufs=3))
            pseg = ctx.enter_context(tc.tile_pool(name="pseg", bufs=2, space="PSUM"))
            ptr = ctx.enter_context(tc.tile_pool(name="ptr", bufs=2, space="PSUM"))
            pout = ctx.enter_context(tc.tile_pool(name="pout", bufs=2, space="PSUM"))

            # index images first: the gathers depend only on these
            ilo_sb = const.tile([P, ntl * 8], I16)
            nc.sync.dma_start(out=ilo_sb[:], in_=ilo[:])
            ihi_sb = const.tile([P, nth * 8], I16)
            nc.sync.dma_start(out=ihi_sb[:], in_=ihi[:])

            ident = const.tile([P, P], F32)
            make_identity(nc, ident[:])
            iota_i = const.tile([P, P], mybir.dt.int32)
            nc.gpsimd.iota(iota_i[:], pattern=[[1, P]], base=0, channel_multiplier=0)
            iota_f = const.tile([P, P], F32)
            nc.vector.tensor_copy(iota_f[:], iota_i[:])

            wt_sb = const.tile([C, C], F32)
            nc.sync.dma_start(out=wt_sb[:], in_=wt[:])
            a_sb = const.tile([P, C], F32)
            nc.sync.dma_start(out=a_sb[:], in_=arep[:])
            b_sb = const.tile([P, 1], F32)
            nc.sync.dma_start(out=b_sb[:], in_=bcol[:])
            dst_sb = const.tile([P, ntt], F32)
            nc.sync.dma_start(out=dst_sb[:], in_=dst[:])
            binv_sb = const.tile([P, WPC], F32)
            nc.sync.dma_start(out=binv_sb[:], in_=binv[:])

            # slab rows of x, window-major: xsl[p, w*C + c] = xslab[w*128 + p, c]
            xsl = const.tile([P, WPC * C], F32)
            nc.sync.dma_start(
                out=xsl[:].rearrange("p (w c) -> p w c", c=C),
                in_=xslab.rearrange("(w p) c -> p w c", p=P),
            )

            # attention scores for the slab: w = sigmoid(x . a + b), one col/window
            wraw = const.tile([P, WPC], F32)
            for w in range(WPC):
                prod = wpool.tile([P, C], F32, tag="prod")
                nc.vector.tensor_tensor(
                    prod[:], xsl[:, w * C : (w + 1) * C], a_sb[:],
                    op=mybir.AluOpType.mult,
                )
                nc.vector.tensor_reduce(
                    wraw[:, w : w + 1], prod[:],
                    axis=mybir.AxisListType.X, op=mybir.AluOpType.add,
                )
            wall = const.tile([P, WPC], F32)
            nc.scalar.activation(
                wall[:], wraw[:], mybir.ActivationFunctionType.Sigmoid,
                bias=b_sb[:, 0:1], scale=1.0,
            )

            # zero-fill gather buffers once: trimmed lanes are never written
            # by the DMA, and one-hot zero columns must multiply finite data.
            for _i in range(4):
                gz = gpool.tile([P, tmax * C], BF16, tag="g", name=f"gz{_i}")
                nc.vector.memset(gz[:], 0.0)

            for w in range(WPC):
                rows = min(P, SLAB - w * P)
                tl, th = t_lo[w], t_hi[w]
                tt = tl + th
                g = gpool.tile([P, tmax * C], BF16, tag="g")
                off = 0
                for img, tb, t_half in ((ilo_sb, tb_lo[w], tl), (ihi_sb, tb_hi[w], th)):
                    tab = xbf[:HALF, :] if img is ilo_sb else xbf[HALF:, :]
                    t0 = 0
                    while t0 < t_half:
                        tn = min(MAX_CALL_TILES, t_half - t0)
                        ni = tn * P
                        nc.gpsimd.dma_gather(
                            g[:, (off + t0) * C : (off + t0 + tn) * C].rearrange(
                                "p (t c) -> p t c", c=C
                            ),
                            tab,
                            img[:, (tb + t0) * 8 : (tb + t0 + tn) * 8],
                            ni,
                            ni,
                            C,
                            queue_num=qrr.next(),
                        )
                        t0 += tn
                    off += t_half
                ps = pseg.tile([P, C], F32)
                for t in range(tt):
                    col = tb_tot[w] + t
                    s_t = spool.tile([P, P], BF16, tag="S")
                    nc.vector.tensor_tensor(
                        s_t[:],
                        dst_sb[:, col : col + 1].to_broadcast([P, P]),
                        iota_f[:],
                        op=mybir.AluOpType.is_equal,
                    )
                    nc.tensor.matmul(
                        out=ps[:], lhsT=s_t[:], rhs=g[:, t * C : (t + 1) * C],
                        start=(t == 0), stop=(t == tt - 1),
                    )
                # scale rows by Binv while draining PSUM
                epre = wpool.tile([P, C], F32, tag="epre")
                nc.scalar.activation(
                    epre[:], ps[:], mybir.ActivationFunctionType.Copy,
                    scale=binv_sb[:, w : w + 1],
                )
                pst = ptr.tile([P, P], F32)
                nc.tensor.transpose(pst[:], epre[:], ident[:])
                epret = wpool.tile([P, P], F32, tag="epret")
                nc.scalar.copy(epret[:], pst[:])
                pso = pout.tile([P, C], F32)
                nc.tensor.matmul(
                    out=pso[:], lhsT=epret[:], rhs=wt_sb[:], start=True, stop=True
                )
                ot = opool.tile([P, CT], BF16, tag="ot")
                nc.scalar.copy(ot[:, 0:C], pso[:])
                nc.vector.tensor_copy(ot[:, C : C + 1], wall[:, w : w + 1])
                nc.vector.memset(ot[:, C + 1 : CT], 0.0)
                nc.sync.dma_start(
                    out=eslab[w * P : w * P + rows, :], in_=ot[:rows, :]
                )
    nc.compile()
    return nc


def _phase_b_program(t_lo, t_hi):
    """Edge->node aggregation over the full intermediate table, producing the
    per-core output slab out[slab, C] = Dinv * segsum(ea rows)[:, :C] + bias."""
    t_lo = list(map(int, t_lo))
    t_hi = list(map(int, t_hi))
    cap_lo = [(t + 1) // 2 * 2 for t in t_lo]
    cap_hi = [(t + 1) // 2 * 2 for t in t_hi]
    ntl, nth = sum(cap_lo), sum(cap_hi)
    ntt = sum(t_lo) + sum(t_hi)
    tb_lo = np.concatenate([[0], np.cumsum(cap_lo)[:-1]]).astype(int)
    tb_hi = np.concatenate([[0], np.cumsum(cap_hi)[:-1]]).astype(int)
    tb_tot = np.concatenate([[0], np.cumsum(np.add(t_lo, t_hi))[:-1]]).astype(int)
    tmax = max(tl + th for tl, th in zip(t_lo, t_hi))

    nc = _new_nc()
    ea = nc.dram_tensor("ea", [N_EDGES, CT], BF16, kind="ExternalInput").ap()
    ilo = nc.dram_tensor("ilo", [P, ntl * 8], I16, kind="ExternalInput").ap()
    ihi = nc.dram_tensor("ihi", [P, nth * 8], I16, kind="ExternalInput").ap()
    dst = nc.dram_tensor("dst", [P, ntt], F32, kind="ExternalInput").ap()
    biasr = nc.dram_tensor("biasr", [P, C], F32, kind="ExternalInput").ap()
    outslab = nc.dram_tensor("outslab", [SLAB, C], F32, kind="ExternalOutput").ap()

    qrr = _QueueRR()
    with tile.TileContext(nc) as tc:
        with ExitStack() as ctx:
            const = ctx.enter_context(tc.tile_pool(name="const", bufs=1))
            gpool = ctx.enter_context(tc.tile_pool(name="gather", bufs=4))
            spool = ctx.enter_context(tc.tile_pool(name="onehot", bufs=6))
            wpool = ctx.enter_context(tc.tile_pool(name="work", bufs=3))
            opool = ctx.enter_context(tc.tile_pool(name="out", bufs=3))
            pseg = ctx.enter_context(tc.tile_pool(name="pseg", bufs=2, space="PSUM"))

            ilo_sb = const.tile([P, ntl * 8], I16)
            nc.sync.dma_start(out=ilo_sb[:], in_=ilo[:])
            ihi_sb = const.tile([P, nth * 8], I16)
            nc.sync.dma_start(out=ihi_sb[:], in_=ihi[:])

            iota_i = const.tile([P, P], mybir.dt.int32)
            nc.gpsimd.iota(iota_i[:], pattern=[[1, P]], base=0, channel_multiplier=0)
            iota_f = const.tile([P, P], F32)
            nc.vector.tensor_copy(iota_f[:], iota_i[:])

            bias_sb = const.tile([P, C], F32)
            nc.sync.dma_start(out=bias_sb[:], in_=biasr[:])
            dst_sb = const.tile([P, ntt], F32)
            nc.sync.dma_start(out=dst_sb[:], in_=dst[:])

            for _i in range(4):
                gz = gpool.tile([P, tmax * CT], BF16, tag="g", name=f"gz{_i}")
                nc.vector.memset(gz[:], 0.0)

            for w in range(WPC):
                rows = min(P, SLAB - w * P)
                tl, th = t_lo[w], t_hi[w]
                tt = tl + th
                g = gpool.tile([P, tmax * CT], BF16, tag="g")
                off = 0
                for img, tb, t_half in ((ilo_sb, tb_lo[w], tl), (ihi_sb, tb_hi[w], th)):
                    tab = ea[:HALF, :] if img is ilo_sb else ea[HALF:, :]
                    t0 = 0
                    while t0 < t_half:
                        tn = min(MAX_CALL_TILES, t_half - t0)
                        ni = tn * P
                        nc.gpsimd.dma_gather(
                            g[:, (off + t0) * CT : (off + t0 + tn) * CT].rearrange(
                                "p (t c) -> p t c", c=CT
                            ),
                            tab,
                            img[:, (tb + t0) * 8 : (tb + t0 + tn) * 8],
                            ni,
                            ni,
                            CT,
                            queue_num=qrr.next(),
                        )
                        t0 += tn
                    off += t_half
                ps = pseg.tile([P, C + 4], F32)
                for t in range(tt):
                    col = tb_tot[w] + t
                    s_t = spool.tile([P, P], BF16, tag="S")
                    nc.vector.tensor_tensor(
                        s_t[:],
                        dst_sb[:, col : col + 1].to_broadcast([P, P]),
                        iota_f[:],
                        op=mybir.AluOpType.is_equal,
                    )
                    nc.tensor.matmul(
                        out=ps[:], lhsT=s_t[:], rhs=g[:, t * CT : t * CT + C + 4],
                        start=(t == 0), stop=(t == tt - 1),
                    )
                # Dinv = 1 / max(D, tiny); zero-degree rows have zero sums so
                # huge * 0 = 0 matches the reference's where(D > 0, 1/D, 0).
                dmax = wpool.tile([P, 1], F32, tag="dmax")
                nc.vector.tensor_scalar_max(dmax[:], ps[:, C : C + 1], 1e-30)
                dinv = wpool.tile([P, 1], F32, tag="dinv")
                nc.vector.reciprocal(dinv[:], dmax[:])
                ot = opool.tile([P, C], F32, tag="ot")
                nc.scalar.activation(
                    ot[:], ps[:, 0:C], mybir.ActivationFunctionType.Copy,
                    scale=dinv[:, 0:1],
                )
                nc.vector.tensor_tensor(
                    ot[:], ot[:], bias_sb[:], op=mybir.AluOpType.add
                )
                nc.sync.dma_start(
                    out=outslab[w * P : w * P + rows, :], in_=ot[:rows, :]
                )
    nc.compile()
    return nc


def _program(phase, t_lo, t_hi):
    key = (phase, tuple(t_lo), tuple(t_hi))
    if key not in _PROGRAMS:
        _PROGRAMS[key] = (
            _phase_a_program(t_lo, t_hi)
            if phase == "A"
            else _phase_b_program(t_lo, t_hi)
        )
    return _PROGRAMS[key]


# ----------------------------------------------------------------------------
# Entry point
# ----------------------------------------------------------------------------

def _run(nc, in_maps, label):
    kwargs = {}
    if TRACE:
        kwargs = dict(trace=True, trace_cores=[0])
    res = run_bass_kernel_spmd(nc, in_maps, core_ids=list(range(NCORES)), **kwargs)
    if res.exec_time_ns is not None:
        LAST_EXEC_NS[label] = res.exec_time_ns
    return res.results


def kernel(x, hyperedge_index, attn_w, attn_b, lin_w, bias):
    from ml_dtypes import bfloat16

    x = np.ascontiguousarray(np.asarray(x, dtype=np.float32))
    he = np.asarray(hyperedge_index)
    node_idx = he[0].astype(np.int64)
    edge_idx = he[1].astype(np.int64)
    attn_w = np.asarray(attn_w, dtype=np.float32)
    attn_b = np.asarray(attn_b, dtype=np.float32)
    lin_w = np.asarray(lin_w, dtype=np.float32)
    bias = np.asarray(bias, dtype=np.float32)

    # --- host index preprocessing ------------------------------------------
    a_lo, a_hi, a_img_lo, a_img_hi, a_dst = _plan_phase(edge_idx, node_idx)
    b_lo, b_hi, b_img_lo, b_img_hi, b_dst = _plan_phase(node_idx, edge_idx)

    bdeg = np.bincount(edge_idx, minlength=N_EDGES).astype(np.float32)
    binv_full = np.where(bdeg > 0, 1.0 / np.maximum(bdeg, 1.0), 0.0).astype(
        np.float32
    )
    pad = WPC * P - SLAB
    binv_cores = np.pad(
        binv_full.reshape(NCORES, SLAB), ((0, 0), (0, pad))
    ).reshape(NCORES, WPC, P).transpose(0, 2, 1)  # [NCORES, P, WPC]
    binv_cores = np.ascontiguousarray(binv_cores)

    wt_host = np.ascontiguousarray(lin_w.T)  # [in_ch, out_ch]
    a_rep = np.ascontiguousarray(np.broadcast_to(attn_w.reshape(1, C), (P, C)))
    b_col = np.full((P, 1), float(attn_b.reshape(-1)[0]), np.float32)
    bias_rep = np.ascontiguousarray(np.broadcast_to(bias.reshape(1, C), (P, C)))

    x_bf = np.ascontiguousarray(x.astype(bfloat16))
    xslab_pad = np.zeros((NCORES, WPC * P, C), np.float32)
    xslab_pad[:, :SLAB] = x.reshape(NCORES, SLAB, C)

    # --- phase A: node -> edge ---------------------------------------------
    nc_a = _program("A", a_lo, a_hi)
    in_maps_a = [
        {
            "xbf": x_bf,
            "xslab": xslab_pad[c],
            "ilo": a_img_lo[c],
            "ihi": a_img_hi[c],
            "dst": a_dst[c],
            "binv": binv_cores[c],
            "wt": wt_host,
            "arep": a_rep,
            "bcol": b_col,
        }
        for c in range(NCORES)
    ]
    res_a = _run(nc_a, in_maps_a, "A")
    ea = np.ascontiguousarray(
        np.concatenate([r["eslab"] for r in res_a], axis=0)
    )  # [N_EDGES, CT] bf16

    # --- phase B: edge -> node ---------------------------------------------
    nc_b = _program("B", b_lo, b_hi)
    in_maps_b = [
        {
            "ea": ea,
            "ilo": b_img_lo[c],
            "ihi": b_img_hi[c],
            "dst": b_dst[c],
            "biasr": bias_rep,
        }
        for c in range(NCORES)
    ]
    res_b = _run(nc_b, in_maps_b, "B")
    out = np.concatenate([r["outslab"] for r in res_b], axis=0)
    return np.ascontiguousarray(out.astype(np.float32))


# revision 38
# speedup vs baseline: 1.2541x; 1.0261x over previous
"""Trainium2 Bass kernel for nn_NodeAttention (hypergraph message passing).

Math (reference):
    w      = sigmoid(x @ attn_w.T + attn_b)[:, 0]          # per-edge weight (M == N)
    e_feat = Binv * segsum_by_edge(x[node_idx]) @ lin_w.T  # node -> hyperedge
    D      = segsum_by_node(w[edge_idx])
    out    = Dinv * segsum_by_node(e_feat[edge_idx]) + bias

Distribution: 8 cores; core c owns edge rows [c*6250, (c+1)*6250) for the
node->edge phase and node rows of the same range for the edge->node phase.
Each phase is a row gather (SWDGE dma_gather from a replicated DRAM table)
followed by a one-hot-matmul segment sum over windows of 128 destination
segments. lin_w is applied once per 128-row window after aggregation (matmul
commutes with the segment sum); w is carried as column 128 of the intermediate
table so D falls out of the phase-B segment sum for free.

Key performance structure (all verified against perfetto traces):
  - dma_gather descriptor generation runs on one Q7 core pair selected by
    queue_num; rotating calls over 4 SWDGE queues (each with its own
    1024-descriptor ring) removes the ring-full stalls that dominated the
    single-queue version. A call is capped at 1024 descriptors and 4096
    bytes per destination partition (HW-probed).
  - Gather tables are bf16 (x pre-cast on host; the intermediate table is
    written bf16 by phase A), halving HBM gather traffic.
  - Per-(window, half) tile counts are exact per-run compile-time constants
    (max over the 8 cores); index images are padded with index 0 and the
    pad lanes are discarded by their -1 one-hot columns.

dma_gather uses int16 indices, so each table is split at row 32768 into lo/hi
halves. Host-side work is limited to index preprocessing (partition by
destination, sort, pad), hyperedge degree counts, and a dtype cast of x; all
x-dependent math runs on device.
"""

import os
import sys
from contextlib import ExitStack

import numpy as np

for _p in (
    "/root/.axon_site",
    "/root/.axon_site/_ro/trn_rl_repo",
    "/root/.axon_site/_ro/pypackages",
):
    if os.path.isdir(_p) and _p not in sys.path:
        sys.path.append(_p)

import concourse.bass as bass
import concourse.mybir as mybir
import concourse.tile as tile
from concourse import bacc
from concourse.bass_utils import run_bass_kernel_spmd
from concourse.masks import make_identity

P = 128
N_NODES = 50000
N_EDGES = 50000
C = 128          # feature channels
CT = 256         # intermediate bf16 row: [e_feat(128) | w(1) | pad], 512 bytes
HALF = 32768     # int16 index split point
NCORES = 8
SLAB = N_NODES // NCORES          # 6250 rows owned per core
WPC = (SLAB + P - 1) // P         # 49 windows of 128 destinations per core
NQ = 4                            # SWDGE queues, round-robin per gather call

F32 = mybir.dt.float32
BF16 = mybir.dt.bfloat16
I16 = mybir.dt.int16

# Set by test harness to capture NTFF profiles / exec times.
TRACE = False
LAST_EXEC_NS = {}

_PROGRAMS = {}


# ----------------------------------------------------------------------------
# Host-side index preprocessing
# ----------------------------------------------------------------------------

GRP = 6  # windows whose gathers share one SBUF buffer / call stream


def _group_layout(t):
    """Tile-image layout for one half: within each group of GRP windows the
    windows' tiles are contiguous; group bases are even-rounded.
    Returns (tbase[w] global tile index per window, total image tiles)."""
    t = np.asarray(t, np.int64)
    ex = np.concatenate([[0], np.cumsum(t)])
    tbase = np.zeros(len(t), np.int64)
    base = 0
    for s in range(0, len(t), GRP):
        e = min(s + GRP, len(t))
        tbase[s:e] = base + ex[s:e] - ex[s]
        base += ((int(ex[e] - ex[s]) + 1) // 2) * 2
    return tbase, int(base)


def _plan_phase(dst_ids, src_ids):
    """Group entries by (destination core, 128-dest window, src half); per
    (window, half) the tile count is the max over cores of ceil(count/128)
    and each core's index image is 0-padded past its own count.

    Returns (t_lo, t_hi, img_lo, img_hi, dst):
      t_lo/t_hi: [WPC] int tile counts per window for the lo/hi halves
      img_lo:    [NCORES, P, sum(t_lo) * 8] int16 dma_gather index image
      img_hi:    [NCORES, P, sum(t_hi) * 8] int16 (indices rebased by -HALF)
      dst:       [NCORES, P, sum(t_lo) + sum(t_hi)] bf16 dest-rel, pad -1
    Within window w the gather sequence is lo entries then hi entries; entry
    i lands at SBUF partition i%128 of tile i//128 of its half, and its
    one-hot column lives at dst[:, tile_col].
    """
    dst_ids = np.asarray(dst_ids, np.int64)
    src_ids = np.asarray(src_ids, np.int64)
    core = dst_ids // SLAB
    local = dst_ids - core * SLAB
    w = local // P
    rel = (local - w * P).astype(np.float32)
    hi = (src_ids >= HALF).astype(np.int64)
    key = (core * WPC + w) * 2 + hi
    order = np.argsort(key, kind="stable")
    k = key[order]
    s = src_ids[order]
    r = rel[order]
    n_grp = NCORES * WPC * 2
    counts = np.bincount(k, minlength=n_grp).reshape(NCORES, WPC, 2)
    t_lo = np.ceil(counts[:, :, 0].max(axis=0) / P).astype(np.int64)  # [WPC]
    t_hi = np.ceil(counts[:, :, 1].max(axis=0) / P).astype(np.int64)
    starts = np.cumsum(counts.reshape(-1)) - counts.reshape(-1)
    rank = np.arange(k.shape[0], dtype=np.int64) - starts[k]
    half_flag = k % 2
    gw = k // 2
    cc = gw // WPC
    ww = gw - cc * WPC

    # capacity layout: windows are processed in groups of GRP; within a group
    # each half's tiles are contiguous so gather calls pack to 8 tiles across
    # window boundaries. Group bases are rounded to even so every call's
    # index-image slice starts 32-byte aligned; inter-group pad tiles are
    # never referenced.
    tbase_lo, ntile_lo = _group_layout(t_lo)
    tbase_hi, ntile_hi = _group_layout(t_hi)

    def build_img(sel, tb, ntiles, base):
        # Pad with index 0: trailing -1 "trim" entries abort on HW at scale
        # (probed); row-0 pad gathers are discarded by the -1 one-hot columns.
        cap = ntiles * P
        img_seq = np.zeros((NCORES, cap), np.int16)
        pos = (tb[ww[sel]] * P + rank[sel]).astype(np.int64)
        img_seq[cc[sel], pos] = (s[sel] - base).astype(np.int16)
        # index i -> partition i % 16, column i // 16; replicate x8 partitions
        img = img_seq.reshape(NCORES, cap // 16, 16).transpose(0, 2, 1)
        return np.ascontiguousarray(np.tile(img, (1, 8, 1)))

    img_lo = build_img(half_flag == 0, tbase_lo, ntile_lo, 0)
    img_hi = build_img(half_flag == 1, tbase_hi, ntile_hi, HALF)

    # dst one-hot image: one column per tile (lo tiles then hi tiles, in
    # window-major concatenated layout to match the kernel's column indexing)
    t_tot = t_lo + t_hi
    tbase_tot = np.concatenate([[0], np.cumsum(t_tot)[:-1]])
    dst_img = np.full((NCORES, P, int(t_tot.sum())), -1.0, np.float32)
    lane = rank % P
    tloc = rank // P
    col = np.where(
        half_flag == 0,
        tbase_tot[ww] + tloc,
        tbase_tot[ww] + t_lo[ww] + tloc,
    )
    dst_img[cc, lane, col] = r
    return t_lo, t_hi, img_lo, img_hi, dst_img


# ----------------------------------------------------------------------------
# Bass programs
# ----------------------------------------------------------------------------

def _new_nc():
    return bacc.Bacc(
        "TRN2",
        target_bir_lowering=False,
        debug=False,
        enable_asserts=False,
        num_devices=NCORES,
        num_swdge_queues=NQ,
    )


class _QueueRR:
    def __init__(self):
        self.i = 0

    def next(self):
        q = self.i % NQ
        self.i += 1
        return q


# Per dma_gather call (HW-probed): <= 1024 descriptors AND <= 4096 bytes
# written per destination partition.
MAX_CALL_TILES = 8


def _phase_a_program(t_lo, t_hi):
    """Node->edge aggregation, producing the per-core slab of the
    intermediate table ea[slab, CT] = [Binv * segsum(x rows) @ lin_w.T | w]."""
    t_lo = list(map(int, t_lo))
    t_hi = list(map(int, t_hi))
    tb_lo, ntl = _group_layout(t_lo)
    tb_hi, nth = _group_layout(t_hi)
    ntt = sum(t_lo) + sum(t_hi)
    tb_tot = np.concatenate([[0], np.cumsum(np.add(t_lo, t_hi))[:-1]]).astype(int)
    ex_lo = np.concatenate([[0], np.cumsum(t_lo)]).astype(int)
    ex_hi = np.concatenate([[0], np.cumsum(t_hi)]).astype(int)
    gmax = max(
        (ex_lo[min(s + GRP, WPC)] - ex_lo[s]) + (ex_hi[min(s + GRP, WPC)] - ex_hi[s])
        for s in range(0, WPC, GRP)
    )

    nc = _new_nc()
    xbf = nc.dram_tensor("xbf", [N_NODES, C], BF16, kind="ExternalInput").ap()
    xslab = nc.dram_tensor("xslab", [WPC * P, C], F32, kind="ExternalInput").ap()
    ilo = nc.dram_tensor("ilo", [P, ntl * 8], I16, kind="ExternalInput").ap()
    ihi = nc.dram_tensor("ihi", [P, nth * 8], I16, kind="ExternalInput").ap()
    dst = nc.dram_tensor("dst", [P, ntt], F32, kind="ExternalInput").ap()
    binv = nc.dram_tensor("binv", [P, WPC], F32, kind="ExternalInput").ap()
    wt = nc.dram_tensor("wt", [C, C], F32, kind="ExternalInput").ap()
    arep = nc.dram_tensor("arep", [P, C], F32, kind="ExternalInput").ap()
    bcol = nc.dram_tensor("bcol", [P, 1], F32, kind="ExternalInput").ap()
    eslab = nc.dram_tensor("eslab", [SLAB, CT], BF16, kind="ExternalOutput").ap()

    qrr = _QueueRR()
    with tile.TileContext(nc) as tc:
        with ExitStack() as ctx:
            const = ctx.enter_context(tc.tile_pool(name="const", bufs=1))
            gpool = ctx.enter_context(tc.tile_pool(name="gather", bufs=3))
            spool = ctx.enter_context(tc.tile_pool(name="onehot", bufs=6))
            wpool = ctx.enter_context(tc.tile_pool(name="work", bufs=3))
            opool = ctx.enter_context(tc.tile_pool(name="out", bufs=3))
            pseg = ctx.enter_context(tc.tile_pool(name="pseg", bufs=2, space="PSUM"))
            ptr = ctx.enter_context(tc.tile_pool(name="ptr", bufs=2, space="PSUM"))
            pout = ctx.enter_context(tc.tile_pool(name="pout", bufs=2, space="PSUM"))

            # index images first: the gathers depend only on these
            ilo_sb = const.tile([P, ntl * 8], I16)
            nc.sync.dma_start(out=ilo_sb[:], in_=ilo[:])
            ihi_sb = const.tile([P, nth * 8], I16)
            nc.sync.dma_start(out=ihi_sb[:], in_=ihi[:])

            ident = const.tile([P, P], F32)
            make_identity(nc, ident[:])
            iota_i = const.tile([P, P], mybir.dt.int32)
            nc.gpsimd.iota(iota_i[:], pattern=[[1, P]], base=0, channel_multiplier=0)
            iota_f = const.tile([P, P], F32)
            nc.vector.tensor_copy(iota_f[:], iota_i[:])

            wt_sb = const.tile([C, C], F32)
            nc.sync.dma_start(out=wt_sb[:], in_=wt[:])
            a_sb = const.tile([P, C], F32)
            nc.sync.dma_start(out=a_sb[:], in_=arep[:])
            b_sb = const.tile([P, 1], F32)
            nc.sync.dma_start(out=b_sb[:], in_=bcol[:])
            dst_sb = const.tile([P, ntt], F32)
            nc.sync.dma_start(out=dst_sb[:], in_=dst[:])
            binv_sb = const.tile([P, WPC], F32)
            nc.sync.dma_start(out=binv_sb[:], in_=binv[:])

            # slab rows of x, window-major: xsl[p, w*C + c] = xslab[w*128 + p, c]
            xsl = const.tile([P, WPC * C], F32)
            nc.sync.dma_start(
                out=xsl[:].rearrange("p (w c) -> p w c", c=C),
                in_=xslab.rearrange("(w p) c -> p w c", p=P),
            )

            # attention scores for the slab: w = sigmoid(x . a + b), one col/window
            wraw = const.tile([P, WPC], F32)
            for w in range(WPC):
                prod = wpool.tile([P, C], F32, tag="prod")
                nc.vector.tensor_tensor(
                    prod[:], xsl[:, w * C : (w + 1) * C], a_sb[:],
                    op=mybir.AluOpType.mult,
                )
                nc.vector.tensor_reduce(
                    wraw[:, w : w + 1], prod[:],
                    axis=mybir.AxisListType.X, op=mybir.AluOpType.add,
                )
            wall = const.tile([P, WPC], F32)
            nc.scalar.activation(
                wall[:], wraw[:], mybir.ActivationFunctionType.Sigmoid,
                bias=b_sb[:, 0:1], scale=1.0,
            )

            # grouped gathers: all of a group's lo tiles stream through calls
            # packed to MAX_CALL_TILES across window boundaries, then the hi
            # tiles, into one shared buffer per group.
            for s in range(0, WPC, GRP):
                e = min(s + GRP, WPC)
                nlo = int(ex_lo[e] - ex_lo[s])
                nhi = int(ex_hi[e] - ex_hi[s])
                g = gpool.tile([P, gmax * C], BF16, tag="g")
                for img, tb0, nt, boff, tab in (
                    (ilo_sb, int(tb_lo[s]), nlo, 0, xbf[:HALF, :]),
                    (ihi_sb, int(tb_hi[s]), nhi, nlo, xbf[HALF:, :]),
                ):
                    t0 = 0
                    while t0 < nt:
                        tn = min(MAX_CALL_TILES, nt - t0)
                        ni = tn * P
                        nc.gpsimd.dma_gather(
                            g[
                                :, (boff + t0) * C : (boff + t0 + tn) * C
                            ].rearrange("p (t c) -> p t c", c=C),
                            tab,
                            img[:, (tb0 + t0) * 8 : (tb0 + t0 + tn) * 8],
                            ni,
                            ni,
                            C,
                            queue_num=qrr.next(),
                        )
                        t0 += tn
                for w in range(s, e):
                    rows = min(P, SLAB - w * P)
                    tl, th = t_lo[w], t_hi[w]
                    tt = tl + th
                    lo_off = int(ex_lo[w] - ex_lo[s])
                    hi_off = nlo + int(ex_hi[w] - ex_hi[s])
                    ps = pseg.tile([P, C], F32)
                    for t in range(tt):
                        col = tb_tot[w] + t
                        goff = (lo_off + t) if t < tl else (hi_off + t - tl)
                        s_t = spool.tile([P, P], BF16, tag="S")
                        nc.vector.tensor_tensor(
                            s_t[:],
                            dst_sb[:, col : col + 1].to_broadcast([P, P]),
                            iota_f[:],
                            op=mybir.AluOpType.is_equal,
                        )
                        nc.tensor.matmul(
                            out=ps[:], lhsT=s_t[:],
                            rhs=g[:, goff * C : (goff + 1) * C],
                            start=(t == 0), stop=(t == tt - 1),
                        )
                    # scale rows by Binv while draining PSUM
                    epre = wpool.tile([P, C], F32, tag="epre")
                    nc.scalar.activation(
                        epre[:], ps[:], mybir.ActivationFunctionType.Copy,
                        scale=binv_sb[:, w : w + 1],
                    )
                    pst = ptr.tile([P, P], F32)
                    nc.tensor.transpose(pst[:], epre[:], ident[:])
                    epret = wpool.tile([P, P], F32, tag="epret")
                    nc.scalar.copy(epret[:], pst[:])
                    pso = pout.tile([P, C], F32)
                    nc.tensor.matmul(
                        out=pso[:], lhsT=epret[:], rhs=wt_sb[:], start=True,
                        stop=True,
                    )
                    ot = opool.tile([P, CT], BF16, tag="ot")
                    nc.scalar.copy(ot[:, 0:C], pso[:])
                    nc.vector.tensor_copy(ot[:, C : C + 1], wall[:, w : w + 1])
                    nc.vector.memset(ot[:, C + 1 : CT], 0.0)
                    nc.sync.dma_start(
                        out=eslab[w * P : w * P + rows, :], in_=ot[:rows, :]
                    )
    nc.compile()
    return nc


def _phase_b_program(t_lo, t_hi):
    """Edge->node aggregation over the full intermediate table, producing the
    per-core output slab out[slab, C] = Dinv * segsum(ea rows)[:, :C] + bias."""
    t_lo = list(map(int, t_lo))
    t_hi = list(map(int, t_hi))
    tb_lo, ntl = _group_layout(t_lo)
    tb_hi, nth = _group_layout(t_hi)
    ntt = sum(t_lo) + sum(t_hi)
    tb_tot = np.concatenate([[0], np.cumsum(np.add(t_lo, t_hi))[:-1]]).astype(int)
    ex_lo = np.concatenate([[0], np.cumsum(t_lo)]).astype(int)
    ex_hi = np.concatenate([[0], np.cumsum(t_hi)]).astype(int)
    gmax = max(
        (ex_lo[min(s + GRP, WPC)] - ex_lo[s]) + (ex_hi[min(s + GRP, WPC)] - ex_hi[s])
        for s in range(0, WPC, GRP)
    )

    nc = _new_nc()
    ea = nc.dram_tensor("ea", [N_EDGES, CT], BF16, kind="ExternalInput").ap()
    ilo = nc.dram_tensor("ilo", [P, ntl * 8], I16, kind="ExternalInput").ap()
    ihi = nc.dram_tensor("ihi", [P, nth * 8], I16, kind="ExternalInput").ap()
    dst = nc.dram_tensor("dst", [P, ntt], F32, kind="ExternalInput").ap()
    biasr = nc.dram_tensor("biasr", [P, C], F32, kind="ExternalInput").ap()
    outslab = nc.dram_tensor("outslab", [SLAB, C], F32, kind="ExternalOutput").ap()

    qrr = _QueueRR()
    with tile.TileContext(nc) as tc:
        with ExitStack() as ctx:
            const = ctx.enter_context(tc.tile_pool(name="const", bufs=1))
            gpool = ctx.enter_context(tc.tile_pool(name="gather", bufs=2))
            spool = ctx.enter_context(tc.tile_pool(name="onehot", bufs=6))
            wpool = ctx.enter_context(tc.tile_pool(name="work", bufs=3))
            opool = ctx.enter_context(tc.tile_pool(name="out", bufs=3))
            pseg = ctx.enter_context(tc.tile_pool(name="pseg", bufs=2, space="PSUM"))

            ilo_sb = const.tile([P, ntl * 8], I16)
            nc.sync.dma_start(out=ilo_sb[:], in_=ilo[:])
            ihi_sb = const.tile([P, nth * 8], I16)
            nc.sync.dma_start(out=ihi_sb[:], in_=ihi[:])

            iota_i = const.tile([P, P], mybir.dt.int32)
            nc.gpsimd.iota(iota_i[:], pattern=[[1, P]], base=0, channel_multiplier=0)
            iota_f = const.tile([P, P], F32)
            nc.vector.tensor_copy(iota_f[:], iota_i[:])

            bias_sb = const.tile([P, C], F32)
            nc.sync.dma_start(out=bias_sb[:], in_=biasr[:])
            dst_sb = const.tile([P, ntt], F32)
            nc.sync.dma_start(out=dst_sb[:], in_=dst[:])

            for s in range(0, WPC, GRP):
                e = min(s + GRP, WPC)
                nlo = int(ex_lo[e] - ex_lo[s])
                nhi = int(ex_hi[e] - ex_hi[s])
                g = gpool.tile([P, gmax * CT], BF16, tag="g")
                for img, tb0, nt, boff, tab in (
                    (ilo_sb, int(tb_lo[s]), nlo, 0, ea[:HALF, :]),
                    (ihi_sb, int(tb_hi[s]), nhi, nlo, ea[HALF:, :]),
                ):
                    t0 = 0
                    while t0 < nt:
                        tn = min(MAX_CALL_TILES, nt - t0)
                        ni = tn * P
                        nc.gpsimd.dma_gather(
                            g[
                                :, (boff + t0) * CT : (boff + t0 + tn) * CT
                            ].rearrange("p (t c) -> p t c", c=CT),
                            tab,
                            img[:, (tb0 + t0) * 8 : (tb0 + t0 + tn) * 8],
                            ni,
                            ni,
                            CT,
                            queue_num=qrr.next(),
                        )
                        t0 += tn
                for w in range(s, e):
                    rows = min(P, SLAB - w * P)
                    tl, th = t_lo[w], t_hi[w]
                    tt = tl + th
                    lo_off = int(ex_lo[w] - ex_lo[s])
                    hi_off = nlo + int(ex_hi[w] - ex_hi[s])
                    ps = pseg.tile([P, C + 4], F32)
                    for t in range(tt):
                        col = tb_tot[w] + t
                        goff = (lo_off + t) if t < tl else (hi_off + t - tl)
                        s_t = spool.tile([P, P], BF16, tag="S")
                        nc.vector.tensor_tensor(
                            s_t[:],
                            dst_sb[:, col : col + 1].to_broadcast([P, P]),
                            iota_f[:],
                            op=mybir.AluOpType.is_equal,
                        )
                        nc.tensor.matmul(
                            out=ps[:], lhsT=s_t[:],
                            rhs=g[:, goff * CT : goff * CT + C + 4],
                            start=(t == 0), stop=(t == tt - 1),
                        )
                    # Dinv = 1 / max(D, tiny); zero-degree rows have zero
                    # sums so huge * 0 = 0 matches where(D > 0, 1/D, 0).
                    dmax = wpool.tile([P, 1], F32, tag="dmax")
                    nc.vector.tensor_scalar_max(dmax[:], ps[:, C : C + 1], 1e-30)
                    dinv = wpool.tile([P, 1], F32, tag="dinv")
                    nc.vector.reciprocal(dinv[:], dmax[:])
                    ot = opool.tile([P, C], F32, tag="ot")
                    nc.scalar.activation(
                        ot[:], ps[:, 0:C], mybir.ActivationFunctionType.Copy,
                        scale=dinv[:, 0:1],
                    )
                    nc.vector.tensor_tensor(
                        ot[:], ot[:], bias_sb[:], op=mybir.AluOpType.add
                    )
                    nc.sync.dma_start(
                        out=outslab[w * P : w * P + rows, :], in_=ot[:rows, :]
                    )
    nc.compile()
    return nc


def _program(phase, t_lo, t_hi):
    key = (phase, tuple(t_lo), tuple(t_hi))
    if key not in _PROGRAMS:
        _PROGRAMS[key] = (
            _phase_a_program(t_lo, t_hi)
            if phase == "A"
            else _phase_b_program(t_lo, t_hi)
        )
    return _PROGRAMS[key]


# ----------------------------------------------------------------------------
# Entry point
# ----------------------------------------------------------------------------

def _run(nc, in_maps, label):
    kwargs = {}
    if TRACE:
        kwargs = dict(trace=True, trace_cores=[0])
    res = run_bass_kernel_spmd(nc, in_maps, core_ids=list(range(NCORES)), **kwargs)
    if res.exec_time_ns is not None:
        LAST_EXEC_NS[label] = res.exec_time_ns
    return res.results


def kernel(x, hyperedge_index, attn_w, attn_b, lin_w, bias):
    from ml_dtypes import bfloat16

    x = np.ascontiguousarray(np.asarray(x, dtype=np.float32))
    he = np.asarray(hyperedge_index)
    node_idx = he[0].astype(np.int64)
    edge_idx = he[1].astype(np.int64)
    attn_w = np.asarray(attn_w, dtype=np.float32)
    attn_b = np.asarray(attn_b, dtype=np.float32)
    lin_w = np.asarray(lin_w, dtype=np.float32)
    bias = np.asarray(bias, dtype=np.float32)

    # --- host index preprocessing ------------------------------------------
    a_lo, a_hi, a_img_lo, a_img_hi, a_dst = _plan_phase(edge_idx, node_idx)
    b_lo, b_hi, b_img_lo, b_img_hi, b_dst = _plan_phase(node_idx, edge_idx)

    bdeg = np.bincount(edge_idx, minlength=N_EDGES).astype(np.float32)
    binv_full = np.where(bdeg > 0, 1.0 / np.maximum(bdeg, 1.0), 0.0).astype(
        np.float32
    )
    pad = WPC * P - SLAB
    binv_cores = np.pad(
        binv_full.reshape(NCORES, SLAB), ((0, 0), (0, pad))
    ).reshape(NCORES, WPC, P).transpose(0, 2, 1)  # [NCORES, P, WPC]
    binv_cores = np.ascontiguousarray(binv_cores)

    wt_host = np.ascontiguousarray(lin_w.T)  # [in_ch, out_ch]
    a_rep = np.ascontiguousarray(np.broadcast_to(attn_w.reshape(1, C), (P, C)))
    b_col = np.full((P, 1), float(attn_b.reshape(-1)[0]), np.float32)
    bias_rep = np.ascontiguousarray(np.broadcast_to(bias.reshape(1, C), (P, C)))

    x_bf = np.ascontiguousarray(x.astype(bfloat16))
    xslab_pad = np.zeros((NCORES, WPC * P, C), np.float32)
    xslab_pad[:, :SLAB] = x.reshape(NCORES, SLAB, C)

    # --- phase A: node -> edge ---------------------------------------------
    nc_a = _program("A", a_lo, a_hi)
    in_maps_a = [
        {
            "xbf": x_bf,
            "xslab": xslab_pad[c],
            "ilo": a_img_lo[c],
            "ihi": a_img_hi[c],
            "dst": a_dst[c],
            "binv": binv_cores[c],
            "wt": wt_host,
            "arep": a_rep,
            "bcol": b_col,
        }
        for c in range(NCORES)
    ]
    res_a = _run(nc_a, in_maps_a, "A")
    ea = np.ascontiguousarray(
        np.concatenate([r["eslab"] for r in res_a], axis=0)
    )  # [N_EDGES, CT] bf16

    # --- phase B: edge -> node ---------------------------------------------
    nc_b = _program("B", b_lo, b_hi)
    in_maps_b = [
        {
            "ea": ea,
            "ilo": b_img_lo[c],
            "ihi": b_img_hi[c],
            "dst": b_dst[c],
            "biasr": bias_rep,
        }
        for c in range(NCORES)
    ]
    res_b = _run(nc_b, in_maps_b, "B")
    out = np.concatenate([r["outslab"] for r in res_b], axis=0)
    return np.ascontiguousarray(out.astype(np.float32))
